# revision 1
# baseline (speedup 1.0000x reference)
"""Trainium2 Bass kernel for nn_AgeGAT (2-layer GAT + global mean pool + linear).

Sharding (8 NeuronCores, SPMD — one program, per-core data):
  launch 1: nodes in 8 equal 128-aligned ranges. Each core builds the full
    layer-1 gather table (h1 = x@W1 + exp-score factors; replicated compute,
    it's cheap), then runs GAT layer 1 for its destination range and writes
    its shard of the layer-2 table.
  host: concatenates layer-2 table shards (pure data movement).
  launch 2: nodes re-sharded on graph boundaries (32 graphs/core). GAT
    layer 2 + per-graph mean pooling + final linear; outputs [32] per core,
    host concatenates to [256].

Edge processing: edges (with self-loops) are sorted by destination, grouped
per 128-node destination block, and within a block by source bank (4 banks
of 25600 rows so dma_gather's int16 indices reach the whole table). Edge
tiles are 128 edges on partitions. Per tile, a one-hot (edge,node) fp16
matrix built by compare-vs-iota drives:
  - numerator/denominator aggregation:  psum[n, 0:nh]   += onehot.T @ ex
                                        psum[n, nh:..]  += onehot.T @ (ex*h)
  - per-edge a_d materialization:       ad_e = onehotT.T @ a_d_block
leaky-relu inside the softmax exp is exact via
  exp(lrelu(as+ad)) = max(exp(as)exp(ad), exp(.2 as)exp(.2 ad)),
with exp(as), exp(.2 as) precomputed per node in the gather rows.
ELU is computed in "v-form" (elu+1); the -1 is folded into the next
layer's constants (W2 column sums / final bias).
"""

import math
import sys
from contextlib import ExitStack

import numpy as np

sys.path.insert(0, "/opt/trn_rl_repo")

import concourse.bass as bass
import concourse.tile as tile
from concourse import mybir
from concourse.ap import AP
from concourse.bass_utils import run_bass_kernel_spmd
from concourse.masks import make_identity
from concourse import library_config

# ---- problem constants ----
N, E, IN, HID, H1, G = 100000, 1600000, 32, 4, 4, 256
IN = 5
HID = 32
NEG = 0.2
P = 128

NCORES = 8
BPC = 100                    # blocks/core (launch 1)
NPC = BPC * P                # 12800 nodes/core
NPAD = NCORES * NPC          # 102400
NBANK = 4
BANKROWS = NPAD // NBANK     # 25600 (int16-addressable)
SBW = 4                      # blocks per superblock (gather batching)

F1 = H1 * HID                # 128
ROW1 = 128                   # table-1 row in f32 elems (512B)
ROW2 = 64                    # table-2 row in f32 elems (256B)
H16OFF1 = 16                 # fp16 elem offset of h in table-1 row
H16OFF2 = 4                  # fp16 elem offset of h3 in table-2 row

FP16 = mybir.dt.float16
F32 = mybir.dt.float32
I16 = mybir.dt.int16
AluOp = mybir.AluOpType
ActFn = mybir.ActivationFunctionType
EPS = 1e-16


# ======================================================================
# host-side preprocessing (integer index/layout work only)
# ======================================================================

def _wrap_idx16(idx):
    """[num] int -> dma_gather idx layout [128, num//16] int16
    (index j at (j%16, j//16), 16-row pattern tiled to 128 partitions)."""
    num = idx.shape[0]
    a = idx.astype(np.int16).reshape(num // 16, 16).T
    return np.tile(a, (8, 1))


def _prep_edges(dst_s, src_s, starts, B):
    """Per-core gather indices + dst_local arrays, blocks sorted by edge
    count (desc) per core so per-position tile counts are tight.
    Returns idx [NCORES,128,TOT], dstl [NCORES,128,TOT], tpos [B], offs [B+1],
    perm [NCORES, B] (position -> original block id)."""
    tc = np.zeros((NCORES, B), np.int64)
    data = []
    for c in range(NCORES):
        lo = np.searchsorted(dst_s, starts[c], side="left")
        hi = np.searchsorted(dst_s, starts[c + 1], side="left")
        d, s = dst_s[lo:hi], src_s[lo:hi]
        blk = (d - starts[c]) // P
        order = np.argsort(blk, kind="stable")
        d, s, blk = d[order], s[order], blk[order]
        cnt = np.bincount(blk, minlength=B)
        tc[c] = (cnt + P - 1) // P
        data.append((d, s, np.concatenate([[0], np.cumsum(cnt)]), cnt))
    perm = np.argsort(-tc, axis=1, kind="stable")      # [NCORES, B]
    tps = np.take_along_axis(tc, perm, axis=1)          # sorted desc per core
    tpos = tps.max(axis=0)                              # [B] position-wise max
    offs = np.concatenate([[0], np.cumsum(tpos)]).astype(np.int64)
    TOT = int(offs[-1])
    idx = np.zeros((NCORES, TOT * P), np.int32)
    dstl = np.full((NCORES, TOT * P), -1.0, np.float32)
    for c in range(NCORES):
        d, s, off_in, cnt = data[c]
        for pos in range(B):
            b = int(perm[c, pos])
            k = int(cnt[b])
            o = int(offs[pos]) * P
            sb = s[off_in[b]:off_in[b + 1]]
            db = d[off_in[b]:off_in[b + 1]]
            so = np.argsort(sb, kind="stable")   # src-sorted tiles
            idx[c, o:o + k] = sb[so]
            dstl[c, o:o + k] = (db[so] - starts[c] - b * P)
    idx3 = idx.reshape(NCORES, TOT, P)
    # per-tile row upper bound across cores (rounded up) for bounded gathers
    hib = idx3.max(axis=(0, 2))
    hib = np.minimum((hib // 4096 + 1) * 4096, NPAD).astype(np.int64)
    return (np.ascontiguousarray(idx3.transpose(0, 2, 1)),
            np.ascontiguousarray(dstl.reshape(NCORES, TOT, P).transpose(0, 2, 1)),
            tpos.astype(np.int64), offs, perm, hib)


def _host_prep(edge_index, batch):
    src = np.asarray(edge_index[0]).astype(np.int64)
    dst = np.asarray(edge_index[1]).astype(np.int64)
    order = np.argsort(dst, kind="stable")
    dst_s, src_s = dst[order], src[order]

    starts1 = np.arange(NCORES + 1, dtype=np.int64) * NPC
    idx1, dstl1, tpos1, offs1, perm1, hib1 = _prep_edges(dst_s, src_s, starts1, BPC)

    b_arr = np.asarray(batch).astype(np.int64)
    gpc = G // NCORES
    starts2 = np.searchsorted(b_arr, np.arange(0, G + 1, gpc)).astype(np.int64)
    starts2[-1] = N
    span = starts2[1:] - starts2[:-1]
    B2 = int(math.ceil(span.max() / P))
    idx2, dstl2, tpos2, offs2, perm2, hib2 = _prep_edges(dst_s, src_s, starts2, B2)

    # batch_local per POSITION (permuted block order)
    batch_local = np.full((NCORES, B2, P), -1.0, np.float32)
    cnts = np.ones((NCORES, 32), np.float32)
    for c in range(NCORES):
        n0, n1 = int(starts2[c]), int(starts2[c + 1])
        bl = np.full(B2 * P, -1.0, np.float32)
        bl[: n1 - n0] = (b_arr[n0:n1] - c * gpc).astype(np.float32)
        batch_local[c] = bl.reshape(B2, P)[perm2[c]]
        cc = np.bincount(b_arr[n0:n1] - c * gpc, minlength=gpc)[:gpc]
        cnts[c] = np.maximum(cc, 1).astype(np.float32)

    return dict(starts1=starts1, idx1=idx1, dstl1=dstl1, tpos1=tpos1,
                offs1=offs1, perm1=perm1, hib1=hib1, hib2=hib2,
                starts2=starts2, idx2=idx2, dstl2=dstl2, tpos2=tpos2,
                offs2=offs2, perm2=perm2, B2=B2,
                batch_local=np.ascontiguousarray(batch_local.transpose(0, 2, 1)),
                cnts=cnts)


# ======================================================================
# AP helpers
# ======================================================================

def bc(ap, n):
    """Append stride-0 axis (broadcast innermost)."""
    return AP(ap.tensor, ap.offset, list(ap.ap) + [[0, n]])


def rep(ap, n):
    """Insert stride-0 axis after partition axis (repeat whole free dim n x)."""
    a = list(ap.ap)
    return AP(ap.tensor, ap.offset, [a[0], [0, n]] + a[1:])


def strided(ap, off, stride, cnt, inner):
    """AP view [P, cnt, inner] with free stride `stride`, elem offset off."""
    return AP(ap.tensor, ap.offset + off, [ap.ap[0], [stride, cnt], [1, inner]])


# ======================================================================
# device builders
# ======================================================================

def legalize_waits(nc, K=1):
    """This walrus encodes at most one semaphore wait per instruction; move
    extra waits onto preceding NoOps on the same engine."""
    n = 0
    for f in nc.m.functions:
        for b in f.blocks:
            newl = []
            changed = False
            for inst in b.instructions:
                si = inst.sync_info
                ow = list(si.on_wait) if si is not None and si.on_wait else []
                if len(ow) > K:
                    changed = True
                    while len(ow) > K:
                        chunk, ow = ow[:K], ow[K:]
                        n += 1
                        newl.append(mybir.InstNoOp(
                            name=f"W-{n}", ins=[], outs=[], engine=inst.engine,
                            sync_info=mybir.SyncInfo(on_wait=chunk, on_update=[])))
                    si.on_wait = ow
                    inst.sync_info = si
                newl.append(inst)
            if changed:
                b.instructions = newl
    return n


def build_launch(layer, meta):
    """layer=1: table build + GAT1 -> shard out. layer=2: GAT2 + pool -> [32]."""
    if layer == 1:
        B, nh, hw, ROW, HOFF = BPC, H1, F1, ROW1, H16OFF1
        tpos, offs = meta["tpos1"], meta["offs1"]
        hib = meta["hib1"]
    else:
        B, nh, hw, ROW, HOFF = meta["B2"], 1, HID, ROW2, H16OFF2
        tpos, offs = meta["tpos2"], meta["offs2"]
        hib = np.full(int(meta["offs2"][-1]), NPAD, np.int64)
    TOT = int(offs[-1])
    TMAX = int(tpos.max())
    rhsw = nh + hw

    nc = bass.Bass()
    ctx = ExitStack()

    # ---- DRAM ----
    idx_d = nc.dram_tensor("idx_d", [P, TOT], mybir.dt.int32, kind="ExternalInput")
    dst_d = nc.dram_tensor("dst_d", [P, TOT], F32, kind="ExternalInput")
    if layer == 1:
        xT = nc.dram_tensor("xT", [IN, NPAD], F32, kind="ExternalInput")
        xloc = nc.dram_tensor("xloc", [IN, NPC], F32, kind="ExternalInput")
        w1 = nc.dram_tensor("w1", [IN, F1], F32, kind="ExternalInput")
        attsrc = nc.dram_tensor("attsrc", [P, F1], F32, kind="ExternalInput")
        attdst = nc.dram_tensor("attdst", [P, F1], F32, kind="ExternalInput")
        b1rep = nc.dram_tensor("b1rep", [P, F1], F32, kind="ExternalInput")
        w2 = nc.dram_tensor("w2", [F1, HID], F32, kind="ExternalInput")
        w2cs = nc.dram_tensor("w2cs", [P, HID], F32, kind="ExternalInput")
        att2s = nc.dram_tensor("att2s", [P, HID], F32, kind="ExternalInput")
        tab = nc.dram_tensor("tab1", [NPAD, ROW1], F32, kind="Internal")
        tabloc = nc.dram_tensor("tabloc", [NPC, ROW1], F32, kind="Internal")
        outT = nc.dram_tensor("out_shard", [NPC, ROW2], F32, kind="ExternalOutput")
    else:
        tab = nc.dram_tensor("tab2", [NPAD, ROW2], F32, kind="ExternalInput")
        tabloc = nc.dram_tensor("tabloc", [B * P, ROW2], F32, kind="ExternalInput")
        att2d = nc.dram_tensor("att2d", [P, HID], F32, kind="ExternalInput")
        b2rep = nc.dram_tensor("b2rep", [P, HID], F32, kind="ExternalInput")
        wlinr = nc.dram_tensor("wlinr", [P, HID], F32, kind="ExternalInput")
        iotag = nc.dram_tensor("iotag", [P, 32], F32, kind="ExternalInput")
        batchl = nc.dram_tensor("batchl", [P, B], F32, kind="ExternalInput")
        cnts = nc.dram_tensor("cnts", [32, 1], F32, kind="ExternalInput")
        blin_d = nc.dram_tensor("blin_d", [32, 1], F32, kind="ExternalInput")
        outT = nc.dram_tensor("out_g", [32, 1], F32, kind="ExternalOutput")
    iotar = nc.dram_tensor("iotar", [P, P], F32, kind="ExternalInput")

    with tile.TileContext(nc) as tc:
        cst = ctx.enter_context(tc.tile_pool(name="const", bufs=1))
        iotaf = cst.tile([P, P], F32)
        nc.sync.dma_start(out=iotaf[:], in_=iotar[:, :])
        iota16 = cst.tile([P, P], FP16)
        nc.vector.tensor_copy(out=iota16[:], in_=iotaf[:])
        ident = cst.tile([P, P], FP16)
        make_identity(nc, ident[:])
        identf = cst.tile([P, P], F32)
        make_identity(nc, identf[:])

        if layer == 1:
            attS = cst.tile([P, F1], F32); nc.sync.dma_start(out=attS[:], in_=attsrc[:, :])
            attD = cst.tile([P, F1], F32); nc.sync.dma_start(out=attD[:], in_=attdst[:, :])
            b1S = cst.tile([P, F1], F32); nc.sync.dma_start(out=b1S[:], in_=b1rep[:, :])
            w2S = cst.tile([F1, HID], F32); nc.sync.dma_start(out=w2S[:], in_=w2[:, :])
            w2cS = cst.tile([P, HID], F32); nc.sync.dma_start(out=w2cS[:], in_=w2cs[:, :])
            a2sS = cst.tile([P, HID], F32); nc.sync.dma_start(out=a2sS[:], in_=att2s[:, :])
            w1S = cst.tile([IN, F1], F32); nc.sync.dma_start(out=w1S[:], in_=w1[:, :])
        else:
            a2dS = cst.tile([P, HID], F32); nc.sync.dma_start(out=a2dS[:], in_=att2d[:, :])
            b2S = cst.tile([P, HID], F32); nc.sync.dma_start(out=b2S[:], in_=b2rep[:, :])
            wlS = cst.tile([P, HID], F32); nc.sync.dma_start(out=wlS[:], in_=wlinr[:, :])
            iogS = cst.tile([P, 32], F32); nc.sync.dma_start(out=iogS[:], in_=iotag[:, :])
            cntS = cst.tile([32, 1], F32); nc.sync.dma_start(out=cntS[:], in_=cnts[:, :])
            blS = cst.tile([32, 1], F32); nc.sync.dma_start(out=blS[:], in_=blin_d[:, :])

        adloc = cst.tile([P, B * nh], FP16)   # local per-node a_d (fp16 rhs)
        # full idx/dst arrays resident in SBUF
        idxS = cst.tile([P, TOT], mybir.dt.int32)
        nc.sync.dma_start(out=idxS[:], in_=idx_d[:, :])
        dstf = cst.tile([P, TOT], F32)
        nc.sync.dma_start(out=dstf[:], in_=dst_d[:, :])
        dstS = cst.tile([P, TOT], FP16)
        nc.vector.tensor_copy(out=dstS[:], in_=dstf[:])

        # ================= phase 1 (layer 1 only): build tables ===========
        if layer == 1:
            # rewritten below with 8-group batches over a flat loop
            with tc.tile_pool(name="h1b", bufs=3) as h1t, \
                 tc.tile_pool(name="h1bps", bufs=2, space="PSUM") as h1p:
                NG = NPAD // P  # 800 groups
                CHG = 40        # groups per xT chunk (800 divisible)
                for chi in range(NG // CHG):
                    xch = h1t.tile([IN, CHG * P], F32, tag="xch")
                    nc.sync.dma_start(out=xch[:],
                                      in_=xT[:, chi * CHG * P:(chi + 1) * CHG * P])
                    for q in range(CHG // 8):
                        stg = h1t.tile([P, 8 * ROW1], F32, tag="stg")
                        hps8 = h1p.tile([P, 8 * F1], F32, tag="hps8")
                        for r in range(8):
                            gi = q * 8 + r
                            nc.tensor.matmul(out=hps8[:, r * F1:(r + 1) * F1],
                                             lhsT=xch[:, gi * P:(gi + 1) * P],
                                             rhs=w1S[:], start=True, stop=True)
                        tmp8 = h1t.tile([P, 8 * F1], F32, tag="tmp8")
                        nc.vector.tensor_tensor(out=tmp8[:], in0=hps8[:],
                                                in1=rep(attS[:], 8), op=AluOp.mult)
                        asf8 = h1t.tile([P, 8 * H1], F32, tag="asf8")
                        nc.vector.tensor_reduce(
                            out=asf8[:], in_=strided(tmp8[:], 0, HID, 8 * H1, HID),
                            axis=mybir.AxisListType.X, op=AluOp.add)
                        nc.scalar.activation(
                            AP(stg[:].tensor, stg[:].offset,
                               [stg[:].ap[0], [ROW1, 8], [1, H1]]),
                            asf8[:], ActFn.Exp)
                        nc.scalar.activation(
                            AP(stg[:].tensor, stg[:].offset + H1,
                               [stg[:].ap[0], [ROW1, 8], [1, H1]]),
                            asf8[:], ActFn.Exp, scale=NEG)
                        s16 = stg[:].bitcast(FP16)
                        nc.vector.tensor_copy(
                            out=AP(s16.tensor, s16.offset + H16OFF1,
                                   [s16.ap[0], [2 * ROW1, 8], [1, F1]]),
                            in_=hps8[:])
                        g0 = (chi * CHG + q * 8) * P
                        nc.sync.dma_start(
                            out=AP(tab[:, :].tensor, g0 * ROW1,
                                   [[ROW1, P], [P * ROW1, 8], [1, ROW1]]),
                            in_=AP(stg[:].tensor, stg[:].offset,
                                   [stg[:].ap[0], [ROW1, 8], [1, ROW1]]))
            # local shard: same row build + a_d, from xloc (per-core data)
            adf = cst.tile([P, B * nh], F32)
            with tc.tile_pool(name="loc", bufs=3) as h1t, \
                 tc.tile_pool(name="locps", bufs=2, space="PSUM") as h1p:
                xl = h1t.tile([IN, NPC], F32, tag="xl")
                nc.sync.dma_start(out=xl[:], in_=xloc[:, :])
                for q in range(B // 4):
                    stg = h1t.tile([P, 4 * ROW1], F32, tag="stg")
                    hps4 = h1p.tile([P, 4 * F1], F32, tag="hps4")
                    for r in range(4):
                        b = q * 4 + r
                        nc.tensor.matmul(out=hps4[:, r * F1:(r + 1) * F1],
                                         lhsT=xl[:, b * P:(b + 1) * P],
                                         rhs=w1S[:], start=True, stop=True)
                    tmp4 = h1t.tile([P, 4 * F1], F32, tag="tmp4")
                    nc.vector.tensor_tensor(out=tmp4[:], in0=hps4[:],
                                            in1=rep(attS[:], 4), op=AluOp.mult)
                    asf4 = h1t.tile([P, 4 * H1], F32, tag="asf4")
                    nc.vector.tensor_reduce(
                        out=asf4[:], in_=strided(tmp4[:], 0, HID, 4 * H1, HID),
                        axis=mybir.AxisListType.X, op=AluOp.add)
                    nc.scalar.activation(
                        AP(stg[:].tensor, stg[:].offset,
                           [stg[:].ap[0], [ROW1, 4], [1, H1]]),
                        asf4[:], ActFn.Exp)
                    nc.scalar.activation(
                        AP(stg[:].tensor, stg[:].offset + H1,
                           [stg[:].ap[0], [ROW1, 4], [1, H1]]),
                        asf4[:], ActFn.Exp, scale=NEG)
                    s16 = stg[:].bitcast(FP16)
                    nc.vector.tensor_copy(
                        out=AP(s16.tensor, s16.offset + H16OFF1,
                               [s16.ap[0], [2 * ROW1, 4], [1, F1]]),
                        in_=hps4[:])
                    tmpd = h1t.tile([P, 4 * F1], F32, tag="tmpd")
                    nc.vector.tensor_tensor(out=tmpd[:], in0=hps4[:],
                                            in1=rep(attD[:], 4), op=AluOp.mult)
                    nc.vector.tensor_reduce(
                        out=adf[:, q * 4 * H1:(q + 1) * 4 * H1],
                        in_=strided(tmpd[:], 0, HID, 4 * H1, HID),
                        axis=mybir.AxisListType.X, op=AluOp.add)
                    g0 = q * 4 * P
                    nc.sync.dma_start(
                        out=AP(tabloc[:, :].tensor, g0 * ROW1,
                               [[ROW1, P], [P * ROW1, 4], [1, ROW1]]),
                        in_=AP(stg[:].tensor, stg[:].offset,
                               [stg[:].ap[0], [ROW1, 4], [1, ROW1]]))
            nc.vector.tensor_copy(out=adloc[:], in_=adf[:])

        # ================= phase 2: edge processing =======================
        with tc.tile_pool(name="slab", bufs=3) as st, \
             tc.tile_pool(name="small", bufs=2) as smt, \
             tc.tile_pool(name="fin", bufs=2) as fnt, \
             tc.tile_pool(name="pstr", bufs=2, space="PSUM") as ptr, \
             tc.tile_pool(name="psad", bufs=1, space="PSUM") as pad_, \
             tc.tile_pool(name="psag", bufs=2, space="PSUM") as pag, \
             tc.tile_pool(name="psfin", bufs=1, space="PSUM") as pfin:

            if layer == 2:
                poolps = pfin.tile([32, 32], F32, tag="poolacc")
                batl = cst.tile([P, B], F32)
                nc.sync.dma_start(out=batl[:], in_=batchl[:, :])

            for b in range(B):
                Tb = int(tpos[b])
                ob = int(offs[b])
                # own-node rows (self loops + layer-2 a_d)
                rows = fnt.tile([P, ROW], F32, tag="rows")
                nc.sync.dma_start(out=rows[:], in_=tabloc[b * P:(b + 1) * P, :])
                r16 = rows[:].bitcast(FP16)
                if layer == 2:
                    adf2 = fnt.tile([P, 1], F32, tag="adf2")
                    tmp2 = fnt.tile([P, HID], F32, tag="adtmp")
                    nc.vector.tensor_tensor(
                        out=tmp2[:],
                        in0=AP(r16.tensor, r16.offset + H16OFF2, [r16.ap[0], [1, HID]]),
                        in1=a2dS[:], op=AluOp.mult)
                    nc.vector.tensor_reduce(
                        out=adf2[:], in_=tmp2[:],
                        axis=mybir.AxisListType.X, op=AluOp.add)
                    nc.vector.tensor_copy(out=adloc[:, b:b + 1], in_=adf2[:])

                if Tb > 0:
                    # gathers: one indirect DMA per tile (128 rows each)
                    slab = st.tile([P, TMAX * ROW], F32, tag="slab")
                    for t in range(Tb):
                        nc.gpsimd.indirect_dma_start(
                            out=slab[:, t * ROW:(t + 1) * ROW],
                            out_offset=None, in_=tab[0:int(hib[ob + t]), :],
                            in_offset=bass.IndirectOffsetOnAxis(
                                ap=idxS[:, ob + t:ob + t + 1], axis=0))

                    # one-hot [e, n] fp16 for all tiles of block
                    oh = smt.tile([P, TMAX * P], FP16, tag="oh")
                    nc.vector.tensor_tensor(
                        out=oh[:, :Tb * P],
                        in0=bc(dstS[:, ob:ob + Tb], P),
                        in1=rep(iota16[:], Tb),
                        op=AluOp.is_equal)
                    # transposed one-hot via PE, batched psum->sbuf copies
                    ohT = smt.tile([P, TMAX * P], FP16, tag="ohT")
                    for q in range((Tb + 3) // 4):
                        tp = ptr.tile([P, 4 * P], FP16, tag="trps")
                        k = min(4, Tb - q * 4)
                        for r in range(k):
                            nc.tensor.transpose(
                                out=tp[:, r * P:(r + 1) * P],
                                in_=oh[:, (q * 4 + r) * P:(q * 4 + r + 1) * P],
                                identity=ident[:])
                        nc.vector.tensor_copy(
                            out=ohT[:, q * 4 * P:(q * 4 + k) * P],
                            in_=tp[:, :k * P])

                    # ad_e for all tiles -> psum [128, Tb*nh]
                    adp = pad_.tile([P, TMAX * nh], F32, tag="adp")
                    for t in range(Tb):
                        nc.tensor.matmul(
                            out=adp[:, t * nh:(t + 1) * nh],
                            lhsT=ohT[:, t * P:(t + 1) * P],
                            rhs=adloc[:, b * nh:(b + 1) * nh],
                            start=True, stop=True)
                    expad = smt.tile([P, TMAX * nh], F32, tag="expad")
                    expad2 = smt.tile([P, TMAX * nh], F32, tag="expad2")
                    nc.scalar.activation(expad[:, :Tb * nh], adp[:, :Tb * nh], ActFn.Exp)
                    nc.scalar.activation(expad2[:, :Tb * nh], adp[:, :Tb * nh],
                                         ActFn.Exp, scale=NEG)

                    # ex = max(A*expad, A2*expad2); A,A2 from slab rows
                    m1 = smt.tile([P, TMAX * nh], F32, tag="m1")
                    ex = smt.tile([P, TMAX * nh], F32, tag="ex")
                    sl = slab[:]
                    nc.vector.tensor_tensor(
                        out=m1[:, :Tb * nh], in0=strided(sl, 0, ROW, Tb, nh),
                        in1=expad[:, :Tb * nh], op=AluOp.mult)
                    nc.vector.tensor_tensor(
                        out=expad2[:, :Tb * nh], in0=strided(sl, nh, ROW, Tb, nh),
                        in1=expad2[:, :Tb * nh], op=AluOp.mult)
                    nc.vector.tensor_tensor(out=ex[:, :Tb * nh], in0=m1[:, :Tb * nh],
                                            in1=expad2[:, :Tb * nh], op=AluOp.max)

                    # rhs fp16 [128, Tb*rhsw]: [ex | ex*h] per tile
                    rhs = smt.tile([P, TMAX * rhsw], FP16, tag="rhs")
                    nc.vector.tensor_copy(
                        out=strided(rhs[:], 0, rhsw, Tb, nh), in_=ex[:, :Tb * nh])
                    sl16 = slab[:].bitcast(FP16)
                    hv = strided(sl16, HOFF, ROW * 2, Tb, hw)
                    exb = AP(rhs[:].tensor, rhs[:].offset,
                             [rhs[:].ap[0], [rhsw, Tb], [1, nh], [0, hw // nh]])
                    nc.vector.tensor_tensor(
                        out=strided(rhs[:], nh, rhsw, Tb, hw),
                        in0=hv, in1=exb, op=AluOp.mult)

                # main aggregation matmuls -> psum [128, rhsw]
                agg = pag.tile([P, rhsw], F32, tag="agg")
                for t in range(Tb):
                    nc.tensor.matmul(
                        out=agg[:],
                        lhsT=oh[:, t * P:(t + 1) * P],
                        rhs=rhs[:, t * rhsw:(t + 1) * rhsw],
                        start=(t == 0), stop=(t == Tb - 1))
                if Tb == 0:
                    nc.vector.memset(agg[:], 0.0)

                # ---- self-loop contribution (from own rows) ----
                sA = fnt.tile([P, nh], F32, tag="sA")
                sexp = fnt.tile([P, nh], F32, tag="sexp")
                sexp2 = fnt.tile([P, nh], F32, tag="sexp2")
                adl32 = fnt.tile([P, nh], F32, tag="adl32")
                nc.vector.tensor_copy(out=adl32[:], in_=adloc[:, b * nh:(b + 1) * nh])
                nc.scalar.activation(sexp[:], adl32[:], ActFn.Exp)
                nc.scalar.activation(sexp2[:], adl32[:], ActFn.Exp, scale=NEG)
                nc.vector.tensor_tensor(out=sexp[:], in0=rows[:, 0:nh],
                                        in1=sexp[:], op=AluOp.mult)
                nc.vector.tensor_tensor(out=sexp2[:], in0=rows[:, nh:2 * nh],
                                        in1=sexp2[:], op=AluOp.mult)
                nc.vector.tensor_tensor(out=sA[:], in0=sexp[:], in1=sexp2[:],
                                        op=AluOp.max)
                # denom += ex_self ; num += ex_self * h_self
                nc.vector.tensor_tensor(out=agg[:, 0:nh], in0=agg[:, 0:nh],
                                        in1=sA[:], op=AluOp.add)
                hself = AP(r16.tensor, r16.offset + HOFF, [r16.ap[0], [1, hw]])
                sh = fnt.tile([P, hw], F32, tag="sh")
                nc.vector.tensor_tensor(
                    out=sh[:], in0=hself,
                    in1=AP(sA[:].tensor, sA[:].offset,
                           [sA[:].ap[0], [1, nh], [0, hw // nh]]),
                    op=AluOp.mult)
                nc.vector.tensor_tensor(out=agg[:, nh:rhsw], in0=agg[:, nh:rhsw],
                                        in1=sh[:], op=AluOp.add)

                # ---- finalize block ----
                den = fnt.tile([P, nh], F32, tag="den")
                nc.vector.tensor_scalar_add(den[:], agg[:, 0:nh], EPS)
                rcp = fnt.tile([P, nh], F32, tag="rcp")
                nc.vector.reciprocal(rcp[:], den[:])
                hv2 = fnt.tile([P, hw], F32, tag="hv2")
                nc.vector.tensor_tensor(
                    out=hv2[:], in0=agg[:, nh:rhsw],
                    in1=AP(rcp[:].tensor, rcp[:].offset,
                           [rcp[:].ap[0], [1, nh], [0, hw // nh]]),
                    op=AluOp.mult)
                yb = fnt.tile([P, hw], F32, tag="yb")
                nc.vector.tensor_tensor(out=yb[:], in0=hv2[:],
                                        in1=(b1S if layer == 1 else b2S)[:, :hw],
                                        op=AluOp.add)
                mn = fnt.tile([P, hw], F32, tag="mn")
                nc.vector.tensor_scalar_min(mn[:], yb[:], 0.0)
                emn = fnt.tile([P, hw], F32, tag="emn")
                nc.scalar.activation(emn[:], mn[:], ActFn.Exp)
                zv = fnt.tile([P, hw], F32, tag="zv")
                nc.vector.scalar_tensor_tensor(
                    out=zv[:], in0=yb[:], scalar=0.0, in1=emn[:],
                    op0=AluOp.max, op1=AluOp.add)

                if layer == 1:
                    tps = pfin.tile([P, P], F32, tag="h2T")
                    nc.tensor.transpose(out=tps[:], in_=zv[:], identity=identf[:])
                    zT = fnt.tile([P, P], F32, tag="zT")
                    nc.vector.tensor_copy(out=zT[:], in_=tps[:])
                    h3p = pfin.tile([P, HID], F32, tag="h3p")
                    nc.tensor.matmul(out=h3p[:], lhsT=zT[:], rhs=w2S[:],
                                     start=True, stop=True)
                    h3 = fnt.tile([P, HID], F32, tag="h3")
                    nc.vector.scalar_tensor_tensor(
                        out=h3[:], in0=h3p[:], scalar=0.0, in1=w2cS[:],
                        op0=AluOp.add, op1=AluOp.subtract)
                    as2 = fnt.tile([P, 1], F32, tag="as2")
                    t3 = fnt.tile([P, HID], F32, tag="t3")
                    nc.vector.tensor_tensor(out=t3[:], in0=h3[:],
                                            in1=a2sS[:], op=AluOp.mult)
                    nc.vector.tensor_reduce(
                        out=as2[:], in_=t3[:],
                        axis=mybir.AxisListType.X, op=AluOp.add)
                    stg2 = fnt.tile([P, ROW2], F32, tag="stg2")
                    nc.scalar.activation(stg2[:, 0:1], as2[:], ActFn.Exp)
                    nc.scalar.activation(stg2[:, 1:2], as2[:], ActFn.Exp, scale=NEG)
                    s216 = stg2[:].bitcast(FP16)
                    nc.vector.tensor_copy(
                        out=AP(s216.tensor, s216.offset + H16OFF2,
                               [s216.ap[0], [1, HID]]),
                        in_=h3[:])
                    nc.sync.dma_start(out=outT[b * P:(b + 1) * P, :], in_=stg2[:])
                else:
                    ohb = fnt.tile([P, 32], F32, tag="ohb")
                    nc.vector.tensor_tensor(
                        out=ohb[:], in0=bc(batl[:, b:b + 1], 32),
                        in1=iogS[:], op=AluOp.is_equal)
                    nc.tensor.matmul(out=poolps[:], lhsT=ohb[:],
                                     rhs=zv[:], start=(b == 0),
                                     stop=(b == B - 1))

            if layer == 2:
                rc = cst.tile([32, 1], F32)
                nc.vector.reciprocal(rc[:], cntS[:])
                pm = cst.tile([32, 32], F32)
                nc.vector.tensor_tensor(out=pm[:], in0=poolps[:],
                                        in1=bc(rc[:], 32), op=AluOp.mult)
                og = cst.tile([32, 1], F32)
                ogs = cst.tile([32, 1], F32)
                tmpo = cst.tile([32, 32], F32)
                nc.vector.tensor_tensor(out=tmpo[:], in0=pm[:],
                                        in1=wlS[:32, :], op=AluOp.mult)
                nc.vector.tensor_reduce(
                    out=ogs[:], in_=tmpo[:],
                    axis=mybir.AxisListType.X, op=AluOp.add)
                nc.vector.tensor_tensor(out=og[:], in0=ogs[:],
                                        in1=blS[:, :1], op=AluOp.add)
                nc.sync.dma_start(out=outT[:, :], in_=og[:])

        ctx.close()
    legalize_waits(nc)
    return nc


# ======================================================================
# entry point
# ======================================================================

LAST_HW_NS = None
PROFILE = False


def _install_ntff_hook():
    import types
    if 'antenv.axon_hooks' in sys.modules:
        return
    mod = types.ModuleType('antenv.axon_hooks')
    mod._hook = None
    mod.set_axon_ntff_profile_hook = lambda h: setattr(mod, '_hook', h)
    mod.get_axon_ntff_profile_hook = lambda: mod._hook
    sys.modules['antenv.axon_hooks'] = mod
    try:
        from trn_agent_boot.trn_boot import _ntff_profile_via_ctypes
        mod.set_axon_ntff_profile_hook(
            _ntff_profile_via_ctypes('/opt/axon/libaxon_pjrt.so'))
    except Exception:
        pass


def _run_retry(nc, in_maps, cores, trace):
    try:
        return run_bass_kernel_spmd(nc, in_maps, cores, trace=trace)
    except Exception:
        import time as _t
        _t.sleep(5)
        return run_bass_kernel_spmd(nc, in_maps, cores, trace=trace)


def kernel(**inputs):
    global LAST_HW_NS
    x = np.asarray(inputs["x"], np.float32)
    W1 = np.asarray(inputs["W1"], np.float32)
    att_src1 = np.asarray(inputs["att_src1"], np.float32)
    att_dst1 = np.asarray(inputs["att_dst1"], np.float32)
    b1 = np.asarray(inputs["b1"], np.float32)
    W2 = np.asarray(inputs["W2"], np.float32)
    att_src2 = np.asarray(inputs["att_src2"], np.float32)
    att_dst2 = np.asarray(inputs["att_dst2"], np.float32)
    b2 = np.asarray(inputs["b2"], np.float32)
    Wlin = np.asarray(inputs["Wlin"], np.float32)
    blin = np.asarray(inputs["blin"], np.float32)
    edge_index = np.asarray(inputs["edge_index"])
    batch = np.asarray(inputs["batch"])

    if PROFILE:
        _install_ntff_hook()

    meta = _host_prep(edge_index, batch)

    xpad = np.zeros((NPAD, IN), np.float32)
    xpad[:N] = x
    xT = np.ascontiguousarray(xpad.T)
    iotar = np.tile(np.arange(P, dtype=np.float32), (P, 1))
    iotag = np.tile(np.arange(32, dtype=np.float32), (P, 1))

    def tile128(v):
        return np.ascontiguousarray(np.tile(v.reshape(1, -1), (P, 1)), dtype=np.float32)

    common1 = dict(
        xT=xT, w1=np.ascontiguousarray(W1),
        attsrc=tile128(att_src1), attdst=tile128(att_dst1),
        b1rep=tile128(b1), w2=np.ascontiguousarray(W2),
        w2cs=tile128(W2.sum(axis=0)), att2s=tile128(att_src2),
        iotar=iotar)
    in_maps1 = []
    for c in range(NCORES):
        xl = xT[:, c * NPC:(c + 1) * NPC].reshape(IN, BPC, P)
        xlp = np.ascontiguousarray(xl[:, meta["perm1"][c]].reshape(IN, NPC))
        in_maps1.append(dict(
            common1,
            xloc=xlp,
            idx_d=np.ascontiguousarray(meta["idx1"][c]),
            dst_d=np.ascontiguousarray(meta["dstl1"][c]),
        ))

    nc1 = build_launch(1, meta)
    res1 = _run_retry(nc1, in_maps1, list(range(NCORES)), PROFILE)
    hw1 = res1.exec_time_ns
    shards = []
    for c in range(NCORES):
        sh = res1.results[c]["out_shard"].reshape(BPC, P, ROW2)
        un = np.empty_like(sh)
        un[meta["perm1"][c]] = sh
        shards.append(un.reshape(NPC, ROW2))
    tab2 = np.concatenate(shards, axis=0)

    B2 = meta["B2"]
    starts2 = meta["starts2"]
    blin_adj = np.float32(blin[0] - Wlin.sum())
    common2 = dict(
        tab2=np.ascontiguousarray(tab2),
        att2d=tile128(att_dst2), b2rep=tile128(b2),
        wlinr=tile128(Wlin[:, 0]), iotag=iotag, iotar=iotar,
        blin_d=np.full((32, 1), blin_adj, np.float32))
    in_maps2 = []
    for c in range(NCORES):
        lo = int(starts2[c])
        tl = np.zeros((B2 * P, ROW2), np.float32)
        real = min(B2 * P, NPAD - lo)
        tl[:real] = tab2[lo:lo + real]
        tl = np.ascontiguousarray(
            tl.reshape(B2, P, ROW2)[meta["perm2"][c]].reshape(B2 * P, ROW2))
        in_maps2.append(dict(
            common2,
            tabloc=tl,
            idx_d=np.ascontiguousarray(meta["idx2"][c]),
            dst_d=np.ascontiguousarray(meta["dstl2"][c]),
            batchl=np.ascontiguousarray(meta["batch_local"][c]),
            cnts=np.ascontiguousarray(meta["cnts"][c].reshape(32, 1)),
        ))

    nc2 = build_launch(2, meta)
    res2 = _run_retry(nc2, in_maps2, list(range(NCORES)), PROFILE)
    hw2 = res2.exec_time_ns
    if hw1 is not None and hw2 is not None:
        LAST_HW_NS = int(hw1) + int(hw2)
    out = np.concatenate([res2.results[c]["out_g"][:, 0] for c in range(NCORES)])
    return out.astype(np.float32)

